# revision 13
# baseline (speedup 1.0000x reference)
"""Multi-head attention (L=2048, EMB=1024, H=16, D=64) on 8 TRN2 NeuronCores.

Tensor-parallel over heads: core i owns heads {2i, 2i+1} (a 128-row block of
Wq/Wk/Wv and a 128-column block of Wo). Each core computes its two heads'
attention plus its partial output projection; the host sums the 8 partials.

Device-side layout is fully transposed (scores^T = [m, l]) so no on-device
transposes are needed:
  QT[d, l] = (Wq_shard @ q^T)        lhsT = (Wq_shard/8)^T, rhs = q^T
  KT[d, l] = (Wk_shard @ k^T)
  V [m, d] = (v @ Wv_shard^T)        lhsT = v^T tile,       rhs = Wv_shard^T
  sT[m, l] = KT_h^T @ QT_h           (per head, contraction d=64)
  sT      += -30 * mask  (fp8 mask applied ON THE PE: one extra matmul per
                          score tile with lhsT = -30*I, rhs = fp8 mask tile,
                          accumulating into the score PSUM bank)
  pT       = exp(sT)                 (masked entries become e^-30*e^s ~ 0)
  attnT/Z  = [V_h | 1]^T @ pT        (ones column gives softmax denominator)
  outT     = Wo_shard^T-block @ (attnT / Z)   partial (bf16), summed on host

Matmuls in bf16 (fp32 PSUM accumulation); mask matmuls in fp8e4.

Performance structure (driven by the PE p-state ramp: the tensor engine only
reaches max clock after ~3us of gap-free execution, so the whole kernel is
organized to keep the PE queue continuously fed):
- warmup matmuls on a zeroed tile ramp the PE from t~7us (after the fixed
  framework preamble) while the first input DMAs land.
- All latency-critical input DMAs are issued on the sync queue in exact
  consumption order (DMA engines drain the global FIFO roughly in issue
  order, so issue order IS priority); only prefetchable bulk (later mask
  chunks, Wo) goes to the gpsimd SWDGE ring.
- Scores run ATTN_LAG quads ahead of the attention matmuls so the PE never
  waits on the exp chain; mask application is on the PE itself, so the only
  cross-engine chain is PE -> scalar(exp) -> PE.
- The softmax denominator never leaves the chip: the Z row of the attn PSUM
  is copied to SBUF (DVE), broadcast across 64 partitions by a K=1 f32r
  matmul into the unused partitions 64:128 of the same PSUM bank, and
  reciprocated by the DVE into SBUF (~3us chain, vs ~8us for a DRAM bounce).
- K/V/Q projection rounds interleave into the first two head-blocks; each
  head's z-chain pieces and each l-tile's output projection drip into the
  following quad streams as PE filler.
"""

import sys

for _p in ("/opt/trn_rl_repo",):
    if _p not in sys.path:
        sys.path.insert(0, _p)

from contextlib import ExitStack

import ml_dtypes
import numpy as np

import concourse.bass as bass
import concourse.tile as tile
from concourse import bacc, mybir
from concourse._compat import with_exitstack
from concourse.bass_utils import run_bass_kernel_spmd

BF16 = mybir.dt.bfloat16
FP8 = mybir.dt.float8e4
F32 = mybir.dt.float32
F32R = mybir.dt.float32r
F16 = mybir.dt.float16
NPBF16 = ml_dtypes.bfloat16
NPFP8 = ml_dtypes.float8_e4m3fn

L = 2048
EMB = 1024
NHEAD = 16
HEAD_DIM = 64
NCORES = 8
HPC = NHEAD // NCORES  # heads per core = 2
ROWS = HPC * HEAD_DIM  # weight rows per core = 128
SCALE = HEAD_DIM ** -0.5

LT = 512               # l-tile (matmul free dim / PSUM bank)
NLT = L // LT          # 4
MT = 128               # m-tile (key-block on partitions)
NMT = L // MT          # 16
ET = 128               # contraction tile over EMB
NET = EMB // ET        # 8
JT = 128               # output-row tile
NJT = EMB // JT        # 8

QUADS = (3, 3, 3, 3, 2, 2)   # m-tiles per exp instruction
QB = 3                        # psc tile m-capacity (PSUM banks per slot)
ATTN_LAG = 3                  # quads the attn matmuls trail the scores
NWARM = 16                    # PE warmup matmuls (p-state ramp)
MASK_NEG = -30.0              # additive mask magnitude (exp(-30+9) ~ 1e-10)


@with_exitstack
def _mha_kernel(ctx, tc, outT, qT, kT, vT, wqT, wkT, wvT, woT, maskT, negIT):
    nc = tc.nc

    const = ctx.enter_context(tc.tile_pool(name="const", bufs=1))
    ppool = ctx.enter_context(tc.tile_pool(name="ptiles", bufs=5))
    maskp = ctx.enter_context(tc.tile_pool(name="maskp", bufs=2))
    stage = ctx.enter_context(tc.tile_pool(name="stage", bufs=3))
    zpool = ctx.enter_context(tc.tile_pool(name="zpool", bufs=3))
    psc = ctx.enter_context(tc.tile_pool(name="psc", bufs=2, space="PSUM"))
    psa = ctx.enter_context(tc.tile_pool(name="psa", bufs=2, space="PSUM"))

    # ---- resident tiles ----
    qTs = const.tile([128, NET, L], BF16, tag="qTs")
    kTs = const.tile([128, NET, L], BF16, tag="kTs")
    vTs = const.tile([128, NET, L], BF16, tag="vTs")
    wqs = const.tile([128, NET, ROWS], BF16, tag="wqs")
    wks = const.tile([128, NET, ROWS], BF16, tag="wks")
    wvs = const.tile([128, NET, ROWS], BF16, tag="wvs")
    wos = const.tile([128, EMB], BF16, tag="wos")  # [hd, j]
    negI = const.tile([128, 128], FP8, tag="negI")
    wz = const.tile([128, 640], BF16, tag="wz")  # warmup zeros
    ones = const.tile([1, 64], F16, tag="ones")  # Z-broadcast lhsT
    q3 = qT.rearrange("(o p) l -> p o l", p=128)
    k3 = kT.rearrange("(o p) l -> p o l", p=128)
    v3 = vT.rearrange("(o p) l -> p o l", p=128)
    mask3 = maskT.rearrange("h (mo p) l -> h p mo l", p=128)

    state = {}

    def mask_fetch(lt, h, eng):
        mc = maskp.tile([128, NMT, LT], FP8, tag="maskc", name=f"maskc_{lt}_{h}")
        eng.dma_start(mc[:], mask3[h, :, :, bass.ts(lt, LT)])
        state[lt, h, "maskc"] = mc

    def chunk_dma(eng, dst, src3, lc):
        eng.dma_start(dst[:, :, bass.ts(lc, LT)], src3[:, :, bass.ts(lc, LT)])

    # warmup inputs: zeroed by gpsimd/vector at t=0 (no dependencies)
    nc.gpsimd.memset(wz[:], 0.0)
    nc.vector.memset(ones[:], 1.0)
    # Input DMAs spread across the three queues, consumption-ordered within
    # each, so aggregate arrival order tracks consumption order:
    #   sync:   Q then K path      scalar: V path      ring: mask + rest
    nc.sync.dma_start(wqs[:], wqT[:])
    chunk_dma(nc.sync, qTs, q3, 0)
    nc.sync.dma_start(wks[:], wkT[:])
    chunk_dma(nc.sync, kTs, k3, 0)
    for lc in (1, 2, 3):
        chunk_dma(nc.sync, kTs, k3, lc)
    for lc in (1, 2, 3):
        chunk_dma(nc.sync, qTs, q3, lc)
    nc.scalar.dma_start(wvs[:], wvT[:])
    for lc in (0, 1, 2, 3):
        chunk_dma(nc.scalar, vTs, v3, lc)
    nc.gpsimd.dma_start(negI[:], negIT[:])
    mask_fetch(0, 0, nc.gpsimd)
    mask_fetch(0, 1, nc.gpsimd)
    nc.gpsimd.dma_start(wos[:], woT[:])

    QTb = const.tile([128, L], BF16, tag="QTb")
    KTb = const.tile([128, L], BF16, tag="KTb")
    VROW = 66
    vaug = const.tile([128, HPC, NMT, VROW], BF16, tag="vaug")
    nc.vector.memset(vaug[:, :, :, HEAD_DIM : HEAD_DIM + 1], 1.0)
    nc.vector.memset(vaug[:, :, :, HEAD_DIM + 1 : VROW], 0.0)

    # ---- PE warmup: ramp the p-state from t~7us on zero data ----
    psw = psc.tile([128, QB, LT], F32, tag="psc", name="ps_warm")
    for i in range(NWARM):
        nc.tensor.matmul(
            psw[:, i % 2, :], lhsT=wz[:, :128], rhs=wz[:, 128:640],
            start=True, stop=True,
        )

    def qk_proj(dst, w, x, lt):
        ps = psc.tile([128, QB, LT], F32, tag="psc", name="ps_proj")[:, 0, :]
        for et in range(NET):
            nc.tensor.matmul(
                ps[:],
                lhsT=w[:, et, :],
                rhs=x[:, et, bass.ts(lt, LT)],
                start=(et == 0),
                stop=(et == NET - 1),
            )
        nc.vector.tensor_copy(out=dst[:, bass.ts(lt, LT)], in_=ps[:])

    def v_proj_tri(mt0, n):
        # n (<=3) m-tiles of the V projection into one psc buf; one copy out
        ps = psc.tile([128, QB, LT], F32, tag="psc", name="ps_v")
        for i in range(n):
            for et in range(NET):
                nc.tensor.matmul(
                    ps[:, i, :ROWS],
                    lhsT=vTs[:, et, bass.ts(mt0 + i, MT)],
                    rhs=wvs[:, et, :],
                    start=(et == 0),
                    stop=(et == NET - 1),
                )
        # ps[:, i, h*64:(h+1)*64] -> vaug[:, h, mt0+i, 0:64]
        src = ps[:, 0:n, :ROWS].rearrange("p n (h d) -> p n h d", h=HPC)
        dst = vaug[:, :, mt0 : mt0 + n, 0:HEAD_DIM].rearrange(
            "p h n d -> p n h d"
        )
        nc.vector.tensor_copy(out=dst, in_=src)

    # ---- attention + per-head on-chip z-chain + per-l-tile epilogue ----
    attnTb = const.tile([128, L], BF16, tag="attnTb")

    # Deferred epilogue pieces drip into the next quad streams as PE filler.
    pending = []

    def piece_zbcast(lt, h, pa, zrow):
        def go():
            # replicate Z across partitions 64:128 of pa's own PSUM bank
            nc.tensor.matmul(
                pa[HEAD_DIM : HEAD_DIM + 64, :],
                lhsT=ones[:],
                rhs=zrow[:],
                start=True,
                stop=True,
            )
        return go

    def piece_recip(lt, h, pa):
        def go():
            hd = bass.ts(h, HEAD_DIM)
            zinv = zpool.tile([128, LT], F32, tag="zinv", name=f"zinv_{lt}_{h}")
            nc.vector.reciprocal(zinv[hd, :], pa[HEAD_DIM : HEAD_DIM + 64, :])
            state[lt, h, "zinv"] = zinv
        return go

    def piece_norm(lt, h):
        def go():
            hd = bass.ts(h, HEAD_DIM)
            sl = attnTb[hd, bass.ts(lt, LT)]
            nc.vector.tensor_mul(out=sl, in0=sl, in1=state[lt, h, "zinv"][hd, :])
        return go

    def piece_outproj(lt, g, gn):
        # one group: gn (<=3) jt-row-blocks: matmuls into one psc buf, one
        # grouped bf16 cast, one ring store
        def go():
            ls = bass.ts(lt, LT)
            ps = psc.tile([128, QB, LT], F32, tag="psc", name="ps_out")
            for i in range(gn):
                nc.tensor.matmul(
                    ps[:, i, :],
                    lhsT=wos[:, bass.ts(3 * g + i, JT)],
                    rhs=attnTb[:, ls],
                    start=True,
                    stop=True,
                )
            st = stage.tile([128, QB, LT], BF16, tag="st", name="st")
            nc.vector.tensor_copy(out=st[:, 0:gn, :], in_=ps[:, 0:gn, :])
            dst = outT[3 * g * JT : (3 * g + gn) * JT, ls].rearrange(
                "(n p) l -> p n l", p=128
            )
            nc.gpsimd.dma_start(dst, st[:, 0:gn, :])
        return go

    def drip():
        if pending:
            pending[0][0] -= 1
            if pending[0][0] < 0:
                pending.pop(0)[1]()

    qk_proj(QTb, wqs, qTs, 0)
    qk_proj(KTb, wks, kTs, 0)

    # carried attn-emission queue (lag software pipeline on the PE)
    attnq = []

    for lt in range(NLT):
        ls = bass.ts(lt, LT)
        for h in range(HPC):
            hd = bass.ts(h, HEAD_DIM)
            nxt = lt * HPC + h + 2  # prefetch two head-blocks ahead
            if nxt < NLT * HPC:
                mask_fetch(nxt // HPC, nxt % HPC, nc.gpsimd)
            maskc = state[lt, h, "maskc"]
            pa = psa.tile([128, LT], F32, tag="psa", name=f"psa_{lt}_{h}")
            mt0 = 0
            for qi, qn in enumerate(QUADS):
                # projection fillers run before their first consumers
                if lt == 0 and h == 0:
                    if 1 <= qi <= 3:
                        qk_proj(KTb, wks, kTs, qi)
                    if 1 <= qi <= 5:
                        v_proj_tri(3 * (qi - 1), 3)
                elif lt == 0 and h == 1:
                    if qi == 0:
                        v_proj_tri(15, 1)
                    elif 1 <= qi <= 3:
                        qk_proj(QTb, wqs, qTs, qi)
                # scores + fp8 mask-add for this quad
                ss = psc.tile([128, QB, LT], F32, tag="psc", name="ss")
                for i in range(qn):
                    nc.tensor.matmul(
                        ss[:, i, :],
                        lhsT=KTb[hd, bass.ts(mt0 + i, MT)],
                        rhs=QTb[hd, ls],
                        start=True,
                        stop=False,
                    )
                for i in range(qn):
                    nc.tensor.matmul(
                        ss[:, i, :],
                        lhsT=negI[:],
                        rhs=maskc[:, mt0 + i, :],
                        start=False,
                        stop=True,
                    )
                pT = ppool.tile([128, QB, LT], BF16, tag="pT", name="pT")
                nc.scalar.activation(
                    pT[:, :qn, :], ss[:, :qn, :], mybir.ActivationFunctionType.Exp
                )

                def make_attn(mt0=mt0, qn=qn, pT=pT, pa=pa, h=h):
                    def go():
                        for i in range(qn):
                            mt = mt0 + i
                            nc.tensor.matmul(
                                pa[:VROW, :],
                                lhsT=vaug[:, h, mt, :],
                                rhs=pT[:, i, :],
                                start=(mt == 0),
                                stop=(mt == NMT - 1),
                            )
                    return go

                attnq.append(make_attn())

                drip()

                # lagged drain of the attn pipeline
                while len(attnq) > ATTN_LAG:
                    attnq.pop(0)()
                mt0 += qn

            # head epilogue: attnT + Z-row copies (DVE; behind the last attn),
            # then the on-chip z-chain drips into the next head's stream.
            # The last head of the l-tile also queues the output projection
            # (strictly after both norms in the pending queue).
            def head_copies(lt=lt, h=h, pa=pa, hd=hd, ls=ls):
                def go():
                    zrow = zpool.tile([1, LT], F16, tag="zrow",
                                      name=f"zrow_{lt}_{h}")
                    nc.vector.tensor_copy(out=attnTb[hd, ls],
                                          in_=pa[0:HEAD_DIM, :])
                    nc.vector.tensor_copy(
                        out=zrow[:], in_=pa[HEAD_DIM : HEAD_DIM + 1, :]
                    )
                    pending.insert(0, [0, piece_zbcast(lt, h, pa, zrow)])
                    pending.insert(1, [0, piece_recip(lt, h, pa)])
                    pending.insert(2, [0, piece_norm(lt, h)])
                    if h == HPC - 1:
                        pending.append([1, piece_outproj(lt, 0, 3)])
                        pending.append([1, piece_outproj(lt, 1, 3)])
                        pending.append([1, piece_outproj(lt, 2, 2)])
                return go
            attnq.append(head_copies())

    while attnq:
        attnq.pop(0)()
    while pending:
        pending.pop(0)[1]()


_CACHE = {}


def _build():
    if "nc" in _CACHE:
        return _CACHE["nc"]
    nc = bacc.Bacc("TRN2", target_bir_lowering=False, debug=False,
                   num_devices=NCORES)
    qT = nc.dram_tensor("qT", [EMB, L], BF16, kind="ExternalInput").ap()
    kT = nc.dram_tensor("kT", [EMB, L], BF16, kind="ExternalInput").ap()
    vT = nc.dram_tensor("vT", [EMB, L], BF16, kind="ExternalInput").ap()
    wqT = nc.dram_tensor("wqT", [128, NET, ROWS], BF16, kind="ExternalInput").ap()
    wkT = nc.dram_tensor("wkT", [128, NET, ROWS], BF16, kind="ExternalInput").ap()
    wvT = nc.dram_tensor("wvT", [128, NET, ROWS], BF16, kind="ExternalInput").ap()
    woT = nc.dram_tensor("woT", [ROWS, EMB], BF16, kind="ExternalInput").ap()
    maskT = nc.dram_tensor("maskT", [HPC, L, L], FP8, kind="ExternalInput").ap()
    negIT = nc.dram_tensor("negIT", [128, 128], FP8, kind="ExternalInput").ap()
    outT = nc.dram_tensor("outT", [EMB, L], BF16, kind="ExternalOutput").ap()

    with tile.TileContext(nc) as tc:
        _mha_kernel(tc, outT, qT, kT, vT, wqT, wkT, wvT, woT, maskT, negIT)
    nc.compile()
    _CACHE["nc"] = nc
    return nc


def _pack_w(w):
    # [ROWS, EMB] -> w.T [EMB, ROWS] -> [128, NET, ROWS] with e = o*128+p
    return np.ascontiguousarray(
        w.T.reshape(NET, 128, ROWS).transpose(1, 0, 2)
    ).astype(NPBF16)


_NEGI = (MASK_NEG * np.eye(128, dtype=np.float32)).astype(NPFP8)


def _prep_in_maps(q, k, v, mask, Wq, Wk, Wv, Wo):
    qT = np.ascontiguousarray(q.T).astype(NPBF16)
    kT = np.ascontiguousarray(k.T).astype(NPBF16)
    vT = np.ascontiguousarray(v.T).astype(NPBF16)
    in_maps = []
    for c in range(NCORES):
        rows = slice(c * ROWS, (c + 1) * ROWS)
        # fp8 {0,1} mask, 1 = masked: bytes 0x00 / 0x38 (= fp8e4m3 1.0)
        mT = np.ascontiguousarray(
            mask[c * HPC : (c + 1) * HPC].swapaxes(1, 2)
        ).view(np.uint8) * np.uint8(0x38)
        in_maps.append({
            "qT": qT,
            "kT": kT,
            "vT": vT,
            "wqT": _pack_w(Wq[rows] * SCALE),
            "wkT": _pack_w(Wk[rows]),
            "wvT": _pack_w(Wv[rows]),
            "woT": np.ascontiguousarray(Wo[:, rows].T).astype(NPBF16),
            "maskT": mT.view(NPFP8),
            "negIT": _NEGI,
        })
    return in_maps


def run(q, k, v, mask, Wq, Wk, Wv, Wo, **spmd_kwargs):
    nc = _build()
    in_maps = _prep_in_maps(q, k, v, mask, Wq, Wk, Wv, Wo)
    res = run_bass_kernel_spmd(nc, in_maps, list(range(NCORES)), **spmd_kwargs)
    outT = np.zeros((EMB, L), np.float32)
    for r in res.results:
        outT += np.asarray(r["outT"], dtype=np.float32)
    out = np.ascontiguousarray(outT.T)
    return out, res


def kernel(q, k, v, mask, Wq, Wk, Wv, Wo):
    q, k, v = (np.asarray(x, np.float32) for x in (q, k, v))
    Wq, Wk, Wv, Wo = (np.asarray(x, np.float32) for x in (Wq, Wk, Wv, Wo))
    mask = np.asarray(mask, bool)
    out, _ = run(q, k, v, mask, Wq, Wk, Wv, Wo)
    return out
